# revision 8
# baseline (speedup 1.0000x reference)
"""Trainium2 Bass kernel for nn_DeepseekLayer (dense transformer layer).

Sharding (8 cores): Megatron-style TP.
  - attention: head-sharded (2 heads/core); q/k/v projections over head shards;
    transposed-softmax layout (scores [sk, sq]); AllToAll switches attention
    output to token shards so o_proj needs no all-reduce.
  - o_proj + residual + rmsnorm2: token-sharded (256 tokens/core).
  - MLP: AllGather hidden -> tensor-parallel gate/up/down (1024 ff dims/core)
    -> ReduceScatter -> local residual add -> host gathers token shards.
All heavy matmuls run in float32r (fp32 bits rounded to 11 mantissa bits,
1 PE cycle/row). Weights are pre-transposed/pre-tiled/pre-rounded on host.
"""
import numpy as np
from contextlib import ExitStack

from concourse import bacc
import concourse.tile as tile
import concourse.mybir as mybir
from concourse.bass_utils import run_bass_kernel_spmd

F32 = mybir.dt.float32
F32R = mybir.dt.float32r
AF = mybir.ActivationFunctionType
OP = mybir.AluOpType

H = 2048          # hidden
NH = 16           # heads
HD = 128          # head dim
MLP = 8192
S = 2048          # sequence
B = 1
EPS = 1e-6
NC = 8            # cores
HPC = NH // NC    # heads per core = 2
EH = HPC * HD     # qkv out dims per core = 256
MSH = MLP // NC   # mlp dims per core = 1024
SSH = S // NC     # tokens per shard = 256
RG = [list(range(NC))]
DT = H // 128     # 16 d-tiles
MT = MSH // 128   # 8 m-tiles per core


def round_fp32r(x: np.ndarray) -> np.ndarray:
    """Round fp32 to fp32r (11 mantissa bits, RNE) — matches walrus fp32_to_fp32r."""
    u = np.ascontiguousarray(x, dtype=np.float32).view(np.uint32).astype(np.uint64)
    r = (u + 0x7FF + ((u >> 12) & 1)) & 0xFFFFF000
    return r.astype(np.uint32).view(np.float32)


def _build_program():
    nc = bacc.Bacc(trn_type="TRN2", target_bir_lowering=False, debug=False,
                   num_devices=NC)

    def inp(name, shape, dt):
        return nc.dram_tensor(name, shape, dt, kind="ExternalInput").ap()

    xT = inp("xT", [H, S], F32)                 # x transposed (feature-major)
    xTs = inp("xTs", [H, SSH], F32)             # this core's token-shard of xT
    cosT = inp("cosT", [HD, S], F32)
    sinT = inp("sinT", [HD, S], F32)
    wqT = inp("wqT", [H, EH], F32R)             # (wq*n1w/sqrt(HD)).T shard
    wkT = inp("wkT", [H, EH], F32R)             # (wk*n1w).T shard
    wvT = inp("wvT", [H, EH], F32R)             # (wv*n1w).T shard
    woTt = inp("woTt", [128, DT, DT, 128], F32R)   # wo.T tiled [p, et, dt, c]
    wgTt = inp("wgTt", [128, DT, MT, 128], F32R)   # (wg*n2w).T shard tiled [p, dt, mt, c]
    wuTt = inp("wuTt", [128, DT, MT, 128], F32R)
    wdTt = inp("wdTt", [128, MT, DT, 128], F32R)   # wd shard.T tiled [p, mt, dt, c]
    out_sh = nc.dram_tensor("out_sh", [H, SSH], F32, kind="ExternalOutput").ap()

    with tile.TileContext(nc) as tc, ExitStack() as top:
        dram = top.enter_context(tc.tile_pool(name="dram", bufs=1, space="DRAM"))
        # persistent SBUF (tiny constants only)
        per = top.enter_context(tc.tile_pool(name="per", bufs=1))
        ones_f = per.tile([128, 1], F32)
        nc.gpsimd.memset(ones_f[:], 1.0)
        ones_r = per.tile([128, 1], F32R)
        nc.vector.tensor_copy(ones_r[:], ones_f[:])
        ones_row = per.tile([1, 128], F32)
        nc.gpsimd.memset(ones_row[:], 1.0)
        eps1 = per.tile([1, 1], F32)
        nc.gpsimd.memset(eps1[:], EPS)

        qk_ctx = ExitStack()
        qk_pool = qk_ctx.enter_context(tc.tile_pool(name="qk", bufs=1))
        qr = [qk_pool.tile([128, S], F32R, name=f"qr{h}") for h in range(HPC)]
        kr = [qk_pool.tile([128, S], F32R, name=f"kr{h}") for h in range(HPC)]
        V_sb = qk_pool.tile([128, S // 128, EH], F32R, name="V_sb")
        att = [qk_pool.tile([128, S], F32R, name=f"att{h}") for h in range(HPC)]

        s12_ctx = ExitStack()
        s12 = s12_ctx.enter_context(tc.tile_pool(name="s12", bufs=1))
        rstd_bc = s12.tile([128, S], F32, name="rstd_bc")
        cos_sb = s12.tile([HD, S], F32, name="cos_sb")
        sin_sb = s12.tile([HD, S], F32, name="sin_sb")
        nc.sync.dma_start(cos_sb[:], cosT)
        nc.sync.dma_start(sin_sb[:], sinT)
        wq_sb = s12.tile([128, DT, EH], F32R, name="wq_sb")
        wk_sb = s12.tile([128, DT, EH], F32R, name="wk_sb")
        wv_sb = s12.tile([128, DT, EH], F32R, name="wv_sb")
        for dt in range(DT):
            nc.sync.dma_start(wq_sb[:, dt, :], wqT[dt * 128:(dt + 1) * 128, :])
            nc.sync.dma_start(wk_sb[:, dt, :], wkT[dt * 128:(dt + 1) * 128, :])
            nc.sync.dma_start(wv_sb[:, dt, :], wvT[dt * 128:(dt + 1) * 128, :])

        # ---- S1: rmsnorm1 statistics (feature-major; partition sums via PE) ----
        with tc.tile_pool(name="s1", bufs=1) as s1, \
             tc.tile_pool(name="ps1", bufs=1, space="PSUM") as ps1:
            ss_ps = [ps1.tile([1, 512], F32, name=f"ss{j}") for j in range(4)]
            for ch in range(2):
                for dt in range(DT):
                    xt = s1.tile([128, 1024], F32, tag="xt", bufs=3)
                    nc.sync.dma_start(xt[:], xT[dt * 128:(dt + 1) * 128,
                                                ch * 1024:(ch + 1) * 1024])
                    x2 = s1.tile([128, 1024], F32R, tag="x2", bufs=2)
                    nc.vector.tensor_tensor(out=x2[:], in0=xt[:], in1=xt[:], op=OP.mult)
                    for j in range(2):
                        nc.tensor.matmul(ss_ps[ch * 2 + j][:], ones_r[:],
                                         x2[:, j * 512:(j + 1) * 512],
                                         start=(dt == 0), stop=(dt == DT - 1))
            ssq = s1.tile([1, S], F32, tag="ssq")
            for j in range(4):
                nc.scalar.activation(ssq[:, j * 512:(j + 1) * 512], ss_ps[j][:],
                                     AF.Sqrt, bias=eps1[:], scale=1.0 / H)
            rstd = s1.tile([1, S], F32, tag="rstd")
            nc.vector.reciprocal(rstd[:], ssq[:])
            for j in range(4):
                bc_ps = ps1.tile([128, 512], F32, tag="bc", bufs=2)
                nc.tensor.matmul(bc_ps[:], ones_row[:], rstd[:, j * 512:(j + 1) * 512],
                                 start=True, stop=True)
                nc.vector.tensor_copy(rstd_bc[:, j * 512:(j + 1) * 512], bc_ps[:])

        # ---- S2: h = norm1(x) (f32r), qkv projections + rope + V ----
        with tc.tile_pool(name="s2", bufs=1) as s2, \
             tc.tile_pool(name="ps2", bufs=1, space="PSUM") as ps2:
            for qd in range(4):          # s-quarters of 512
                c0 = qd * 512
                ht = []
                for dt in range(DT):
                    xt = s2.tile([128, 512], F32, tag="xq", bufs=2)
                    nc.sync.dma_start(xt[:], xT[dt * 128:(dt + 1) * 128, c0:c0 + 512])
                    h_t = s2.tile([128, 512], F32R, tag=f"ht{dt}", bufs=1)
                    nc.vector.tensor_tensor(out=h_t[:], in0=xt[:],
                                            in1=rstd_bc[:, c0:c0 + 512], op=OP.mult)
                    ht.append(h_t)
                # q/k for this quarter (feature-major out [hd, s])
                q_ps = [ps2.tile([128, 512], F32, tag=f"q{h}", name=f"q_ps{h}", bufs=1) for h in range(HPC)]
                k_ps = [ps2.tile([128, 512], F32, tag=f"k{h}", name=f"k_ps{h}", bufs=1) for h in range(HPC)]
                for dt in range(DT):
                    for h in range(HPC):
                        nc.tensor.matmul(q_ps[h][:], wq_sb[:, dt, h * 128:(h + 1) * 128],
                                         ht[dt][:], start=(dt == 0), stop=(dt == DT - 1))
                        nc.tensor.matmul(k_ps[h][:], wk_sb[:, dt, h * 128:(h + 1) * 128],
                                         ht[dt][:], start=(dt == 0), stop=(dt == DT - 1))
                # rope into resident qr/kr
                for h in range(HPC):
                    for (src, dst) in ((q_ps[h], qr[h]), (k_ps[h], kr[h])):
                        m1 = s2.tile([64, 512], F32, tag="m1", bufs=1)
                        m2 = s2.tile([64, 512], F32, tag="m2", bufs=1)
                        nc.vector.tensor_tensor(out=m1[:], in0=src[0:64, :],
                                                in1=cos_sb[0:64, c0:c0 + 512], op=OP.mult)
                        nc.vector.tensor_tensor(out=m2[:], in0=src[64:128, :],
                                                in1=sin_sb[0:64, c0:c0 + 512], op=OP.mult)
                        nc.vector.tensor_tensor(out=dst[0:64, c0:c0 + 512], in0=m1[:],
                                                in1=m2[:], op=OP.subtract)
                        m3 = s2.tile([64, 512], F32, tag="m3", bufs=1)
                        m4 = s2.tile([64, 512], F32, tag="m4", bufs=1)
                        nc.vector.tensor_tensor(out=m3[:], in0=src[64:128, :],
                                                in1=cos_sb[64:128, c0:c0 + 512], op=OP.mult)
                        nc.vector.tensor_tensor(out=m4[:], in0=src[0:64, :],
                                                in1=sin_sb[64:128, c0:c0 + 512], op=OP.mult)
                        nc.vector.tensor_tensor(out=dst[64:128, c0:c0 + 512], in0=m3[:],
                                                in1=m4[:], op=OP.add)
                # V token-major [s, e]
                for sti in range(4):
                    st = qd * 4 + sti
                    v_ps = ps2.tile([128, EH], F32, tag="v", bufs=2)
                    for dt in range(DT):
                        nc.tensor.matmul(v_ps[:], ht[dt][:, sti * 128:(sti + 1) * 128],
                                         wv_sb[:, dt, :], start=(dt == 0), stop=(dt == DT - 1))
                    nc.scalar.copy(V_sb[:, st, :], v_ps[:])

        # ---- S3: attention (transposed softmax, no max subtraction) ----
        s12_ctx.close()
        with tc.tile_pool(name="s3", bufs=1) as s3, \
             tc.tile_pool(name="ps3", bufs=1, space="PSUM") as ps3:
            for h in range(HPC):
                for sc in range(4):      # sq chunks of 512
                    q0 = sc * 512
                    av_ps = ps3.tile([128, 512], F32, tag="av", bufs=2)
                    sm_ps = ps3.tile([1, 512], F32, tag="sm", bufs=2)
                    for kt in range(DT):  # sk tiles of 128
                        st_ps = ps3.tile([128, 512], F32, tag="st", bufs=2)
                        nc.tensor.matmul(st_ps[:], kr[h][:, kt * 128:(kt + 1) * 128],
                                         qr[h][:, q0:q0 + 512], start=True, stop=True)
                        e_sb = s3.tile([128, 512], F32R, tag="e", bufs=3)
                        nc.scalar.activation(e_sb[:], st_ps[:], AF.Exp)
                        nc.tensor.matmul(sm_ps[:], ones_r[:], e_sb[:],
                                         start=(kt == 0), stop=(kt == DT - 1))
                        nc.tensor.matmul(av_ps[:], V_sb[:, kt, h * 128:(h + 1) * 128],
                                         e_sb[:], start=(kt == 0), stop=(kt == DT - 1))
                    rs_sb = s3.tile([1, 512], F32, tag="rs", bufs=2)
                    nc.vector.reciprocal(rs_sb[:], sm_ps[:])
                    bc_ps = ps3.tile([128, 512], F32, tag="bc", bufs=1)
                    nc.tensor.matmul(bc_ps[:], ones_row[:], rs_sb[:], start=True, stop=True)
                    bc_sb = s3.tile([128, 512], F32, tag="bcs", bufs=2)
                    nc.vector.tensor_copy(bc_sb[:], bc_ps[:])
                    nc.vector.tensor_tensor(out=att[h][:, q0:q0 + 512], in0=av_ps[:],
                                            in1=bc_sb[:], op=OP.mult)

        # ---- S4: AllToAll to token shards + o_proj + residual ----
        a2a_in = dram.tile([NC, EH, SSH], F32R)
        a2a_out = dram.tile([NC, EH, SSH], F32R)
        for j in range(NC):
            for h in range(HPC):
                nc.sync.dma_start(a2a_in[j, h * 128:(h + 1) * 128, :],
                                  att[h][:, j * SSH:(j + 1) * SSH])
        nc.gpsimd.collective_compute("AllToAll", OP.bypass,
                                     ins=[a2a_in[:]], outs=[a2a_out[:]],
                                     replica_groups=RG)
        qk_ctx.close()
        res_pool = top.enter_context(tc.tile_pool(name="res", bufs=1))
        res1 = [res_pool.tile([128, SSH], F32, name=f"res1_{dt}") for dt in range(DT)]
        with tc.tile_pool(name="s4", bufs=1) as s4, \
             tc.tile_pool(name="ps4", bufs=1, space="PSUM") as ps4:
            attg = s4.tile([128, DT, SSH], F32R, tag="attg")
            for et in range(DT):
                nc.sync.dma_start(attg[:, et, :],
                                  a2a_out[et // 2, (et % 2) * 128:(et % 2) * 128 + 128, :])
            for dt in range(DT):
                wo_t = s4.tile([128, DT, 128], F32R, tag="wo", bufs=2)
                nc.sync.dma_start(wo_t[:], woTt[:, :, dt, :])
                o_ps = ps4.tile([128, SSH], F32, tag="o", bufs=2)
                for et in range(DT):
                    nc.tensor.matmul(o_ps[:], wo_t[:, et, :], attg[:, et, :],
                                     start=(et == 0), stop=(et == DT - 1))
                xs = s4.tile([128, SSH], F32, tag="xs", bufs=2)
                nc.sync.dma_start(xs[:], xTs[dt * 128:(dt + 1) * 128, :])
                nc.vector.tensor_tensor(out=res1[dt][:], in0=o_ps[:], in1=xs[:], op=OP.add)

        # ---- S5: rmsnorm2 on token shard ----
        h2 = [res_pool.tile([128, SSH], F32R, name=f"h2_{dt}") for dt in range(DT)]
        with tc.tile_pool(name="s5", bufs=1) as s5, \
             tc.tile_pool(name="ps5", bufs=1, space="PSUM") as ps5:
            ss2_ps = ps5.tile([1, SSH], F32, tag="ss2")
            for dt in range(DT):
                x2 = s5.tile([128, SSH], F32R, tag="x22", bufs=2)
                nc.vector.tensor_tensor(out=x2[:], in0=res1[dt][:], in1=res1[dt][:], op=OP.mult)
                nc.tensor.matmul(ss2_ps[:], ones_r[:], x2[:],
                                 start=(dt == 0), stop=(dt == DT - 1))
            ssq2 = s5.tile([1, SSH], F32, tag="ssq2")
            nc.scalar.activation(ssq2[:], ss2_ps[:], AF.Sqrt, bias=eps1[:], scale=1.0 / H)
            rstd2 = s5.tile([1, SSH], F32, tag="rstd2")
            nc.vector.reciprocal(rstd2[:], ssq2[:])
            bc2_ps = ps5.tile([128, SSH], F32, tag="bc2", bufs=1)
            nc.tensor.matmul(bc2_ps[:], ones_row[:], rstd2[:], start=True, stop=True)
            rstd2_bc = s5.tile([128, SSH], F32, tag="rstd2bc")
            nc.vector.tensor_copy(rstd2_bc[:], bc2_ps[:])
            for dt in range(DT):
                nc.vector.tensor_tensor(out=h2[dt][:], in0=res1[dt][:],
                                        in1=rstd2_bc[:], op=OP.mult)

        # ---- S6: AllGather hidden shards ----
        ag_in = dram.tile([H, SSH], F32R)
        ag_out = dram.tile([NC, H, SSH], F32R, addr_space="Shared")
        for dt in range(DT):
            nc.sync.dma_start(ag_in[dt * 128:(dt + 1) * 128, :], h2[dt][:])
        nc.gpsimd.collective_compute("AllGather", OP.bypass,
                                     ins=[ag_in[:]], outs=[ag_out[:]],
                                     replica_groups=RG)

        # ---- S7: MLP (TP over ff dims) + S8: ReduceScatter + residual ----
        rs_in = dram.tile([NC, H, SSH], F32)
        with tc.tile_pool(name="s7", bufs=1) as s7, \
             tc.tile_pool(name="ps7", bufs=1, space="PSUM") as ps7:
            for half in range(2):        # s halves of 1024 (2 rank-block pairs)
                # stream hidden for this half: [128, dt, 1024]
                h2g = []
                for dt in range(DT):
                    t = s7.tile([128, 1024], F32R, tag=f"hg{dt}", bufs=1)
                    for k in range(4):
                        r = half * 4 + k
                        nc.sync.dma_start(t[:, k * 256:(k + 1) * 256],
                                          ag_out[r, dt * 128:(dt + 1) * 128, :])
                    h2g.append(t)
                act_t = []
                for mt in range(MT):
                    wg_t = s7.tile([128, DT, 128], F32R, tag="wg", bufs=2)
                    wu_t = s7.tile([128, DT, 128], F32R, tag="wu", bufs=2)
                    nc.sync.dma_start(wg_t[:], wgTt[:, :, mt, :])
                    nc.sync.dma_start(wu_t[:], wuTt[:, :, mt, :])
                    a_t = s7.tile([128, 1024], F32R, tag=f"act{mt}", bufs=1)
                    for sc in range(2):  # 512-chunks within half
                        g_ps = ps7.tile([128, 512], F32, tag="g", bufs=2)
                        u_ps = ps7.tile([128, 512], F32, tag="u", bufs=2)
                        for dt in range(DT):
                            nc.tensor.matmul(g_ps[:], wg_t[:, dt, :],
                                             h2g[dt][:, sc * 512:(sc + 1) * 512],
                                             start=(dt == 0), stop=(dt == DT - 1))
                            nc.tensor.matmul(u_ps[:], wu_t[:, dt, :],
                                             h2g[dt][:, sc * 512:(sc + 1) * 512],
                                             start=(dt == 0), stop=(dt == DT - 1))
                        gs = s7.tile([128, 512], F32, tag="gs", bufs=2)
                        nc.scalar.activation(gs[:], g_ps[:], AF.Sigmoid)
                        nc.vector.tensor_tensor(out=a_t[:, sc * 512:(sc + 1) * 512],
                                                in0=u_ps[:], in1=gs[:], op=OP.mult)
                    act_t.append(a_t)
                # down projection for this half
                for dt in range(DT):
                    wd_t = s7.tile([128, MT, 128], F32R, tag="wd", bufs=2)
                    nc.sync.dma_start(wd_t[:], wdTt[:, :, dt, :])
                    for sc in range(2):
                        d_ps = ps7.tile([128, 512], F32, tag="d", bufs=2)
                        for mt in range(MT):
                            nc.tensor.matmul(d_ps[:], wd_t[:, mt, :],
                                             act_t[mt][:, sc * 512:(sc + 1) * 512],
                                             start=(mt == 0), stop=(mt == MT - 1))
                        dn = s7.tile([128, 512], F32, tag="dn", bufs=3)
                        nc.vector.tensor_copy(dn[:], d_ps[:])
                        r0 = (half * 2 + sc) * 2
                        nc.sync.dma_start(rs_in[r0, dt * 128:(dt + 1) * 128, :],
                                          dn[:, 0:256])
                        nc.sync.dma_start(rs_in[r0 + 1, dt * 128:(dt + 1) * 128, :],
                                          dn[:, 256:512])

        rs_out = dram.tile([H, SSH], F32)
        nc.gpsimd.collective_compute("ReduceScatter", OP.add,
                                     ins=[rs_in[:]], outs=[rs_out[:]],
                                     replica_groups=RG)
        with tc.tile_pool(name="s8", bufs=1) as s8:
            for dt in range(DT):
                rsb = s8.tile([128, SSH], F32, tag="rsb", bufs=3)
                nc.sync.dma_start(rsb[:], rs_out[dt * 128:(dt + 1) * 128, :])
                fin = s8.tile([128, SSH], F32, tag="fin", bufs=3)
                nc.vector.tensor_tensor(out=fin[:], in0=rsb[:], in1=res1[dt][:], op=OP.add)
                nc.sync.dma_start(out_sh[dt * 128:(dt + 1) * 128, :], fin[:])

    nc.compile()
    return nc


_PROG = None


def _get_program():
    global _PROG
    if _PROG is None:
        _PROG = _build_program()
    return _PROG


def _prep_inputs(x, norm1_w, wq, wk, wv, wo, norm2_w, w_gate, w_up, w_down, cos, sin):
    x = np.asarray(x, dtype=np.float32)
    xT = np.ascontiguousarray(x.reshape(S, H).T)                       # [H, S]
    cosT = np.ascontiguousarray(np.asarray(cos, np.float32).T)         # [HD, S]
    sinT = np.ascontiguousarray(np.asarray(sin, np.float32).T)
    n1 = np.asarray(norm1_w, np.float32)
    n2 = np.asarray(norm2_w, np.float32)
    wq = np.asarray(wq, np.float32) * n1[None, :] / np.sqrt(np.float32(HD))
    wk = np.asarray(wk, np.float32) * n1[None, :]
    wv = np.asarray(wv, np.float32) * n1[None, :]
    wg = np.asarray(w_gate, np.float32) * n2[None, :]
    wu = np.asarray(w_up, np.float32) * n2[None, :]
    wo = np.asarray(wo, np.float32)
    wd = np.asarray(w_down, np.float32)

    woT = round_fp32r(wo.T)                                            # [e=H, d=H]
    woTt = np.ascontiguousarray(
        woT.reshape(DT, 128, DT, 128).transpose(1, 0, 2, 3))           # [p, et, dt, c]

    in_maps = []
    for c in range(NC):
        e0 = c * EH
        m0 = c * MSH
        wqT = round_fp32r(wq[e0:e0 + EH, :].T)                         # [H, EH]
        wkT = round_fp32r(wk[e0:e0 + EH, :].T)
        wvT = round_fp32r(wv[e0:e0 + EH, :].T)
        wgT = round_fp32r(wg[m0:m0 + MSH, :].T)                        # [H, MSH]
        wuT = round_fp32r(wu[m0:m0 + MSH, :].T)
        wdT = round_fp32r(wd[:, m0:m0 + MSH].T)                        # [MSH, H]
        in_maps.append({
            "xT": xT,
            "xTs": np.ascontiguousarray(xT[:, c * SSH:(c + 1) * SSH]),
            "cosT": cosT, "sinT": sinT,
            "wqT": np.ascontiguousarray(wqT),
            "wkT": np.ascontiguousarray(wkT),
            "wvT": np.ascontiguousarray(wvT),
            "woTt": woTt,
            "wgTt": np.ascontiguousarray(
                wgT.reshape(DT, 128, MT, 128).transpose(1, 0, 2, 3)),
            "wuTt": np.ascontiguousarray(
                wuT.reshape(DT, 128, MT, 128).transpose(1, 0, 2, 3)),
            "wdTt": np.ascontiguousarray(
                wdT.reshape(MT, 128, DT, 128).transpose(1, 0, 2, 3)),
        })
    return in_maps


def kernel(x, norm1_w, wq, wk, wv, wo, norm2_w, w_gate, w_up, w_down, cos, sin,
           _want_results=False):
    in_maps = _prep_inputs(x, norm1_w, wq, wk, wv, wo, norm2_w,
                           w_gate, w_up, w_down, cos, sin)
    prog = _get_program()
    res = run_bass_kernel_spmd(prog, in_maps, list(range(NC)))
    out = np.empty((B, S, H), dtype=np.float32)
    for c in range(NC):
        out[0, c * SSH:(c + 1) * SSH, :] = res.results[c]["out_sh"].T
    if _want_results:
        return out, res
    return out
